# revision 10
# baseline (speedup 1.0000x reference)
"""Trainium2 Bass kernel: 3-layer LSTM LM (embed -> 3xLSTM(H=256) -> FC 32000 -> log_softmax).

Strategy: data-parallel over batch across 8 cores (2 sequences per core).
Everything else (LSTM recurrence, FC, log_softmax over full vocab) is local
per core; zero collectives.
"""

import sys

sys.path.insert(0, "/opt/trn_rl_repo")

import numpy as np

import concourse.bass as bass
import concourse.mybir as mybir
import concourse.tile as tile
from concourse import bacc
from concourse.bass_utils import run_bass_kernel_spmd
from concourse.masks import make_identity
from concourse.tile import add_dep_helper

# Problem dims
V = 32000
E = 200
H = 256
B = 16
T = 256
N_CORES = 8
B_LOC = B // N_CORES  # 2 sequences per core
G4 = 4 * H  # 1024 gate width

# Tiling
CHUNK = 64  # recurrence steps per xg-precompute chunk
CB = CHUNK * B_LOC  # columns per gate m-chunk in PSUM (128)
N_MCHUNK = G4 // 128  # 8 gate row chunks
TB = T * B_LOC  # 512 (t, b) columns per K-chunk in h buffers
VCHUNK = 512
FP16 = mybir.dt.float16
FP32 = mybir.dt.float32
AF = mybir.ActivationFunctionType
LAYER_DIMS = [E, H, H]


def ksizes(dim):
    """Partition-chunk sizes for a contraction dim."""
    out = []
    while dim > 0:
        out.append(min(dim, 128))
        dim -= 128
    return out


def build_nc(t_steps=T):
    nsteps = t_steps
    nchunks = nsteps // CHUNK if nsteps >= CHUNK else 1
    chunk = min(CHUNK, nsteps)
    cb = chunk * B_LOC
    tb = nsteps * B_LOC
    ntok = nsteps * B_LOC
    n_gtiles = (ntok + 127) // 128

    nc = bacc.Bacc("TRN2", target_bir_lowering=False, debug=False,
                   num_devices=N_CORES)

    # DRAM I/O
    xids_d = nc.dram_tensor("xids", [ntok, 1], mybir.dt.int32, kind="ExternalInput")
    emb_d = nc.dram_tensor("emb", [V, E], FP32, kind="ExternalInput")
    wiT_d = [nc.dram_tensor(f"wiT{l}", [LAYER_DIMS[l], G4], FP16, kind="ExternalInput")
             for l in range(3)]
    whT_d = [nc.dram_tensor(f"whT{l}", [H, G4], FP16, kind="ExternalInput")
             for l in range(3)]
    bvec_d = [nc.dram_tensor(f"bvec{l}", [1, G4], FP16, kind="ExternalInput")
              for l in range(3)]
    fcWT_d = nc.dram_tensor("fcWT", [H, V], FP16, kind="ExternalInput")
    fcb_d = nc.dram_tensor("fcb", [1, V], FP16, kind="ExternalInput")
    out_d = nc.dram_tensor("out", [tb, V], FP32, kind="ExternalOutput")

    with tile.TileContext(nc, num_cores=N_CORES) as tc:
        with (
            tc.tile_pool(name="weights", bufs=1) as wpool,
            tc.tile_pool(name="state", bufs=1) as spool,
            tc.tile_pool(name="work", bufs=3) as work,
            tc.tile_pool(name="psum", bufs=2, space="PSUM") as pp,
            tc.tile_pool(name="fcw", bufs=3) as fcwpool,
            tc.tile_pool(name="fcpsum", bufs=4, space="PSUM") as fcpp,
            tc.tile_pool(name="stage", bufs=3) as stpool,
        ):
            # ---- Phase 0: load weights to SBUF ----
            wiT_sb = []
            whT_sb = []
            bvec_sb = []
            for l in range(3):
                ks = ksizes(LAYER_DIMS[l])
                wi = wpool.tile([128, len(ks) * G4], FP16, tag=f"wiT{l}")
                for kc, ksz in enumerate(ks):
                    nc.sync.dma_start(
                        wi[0:ksz, kc * G4:(kc + 1) * G4],
                        wiT_d[l][kc * 128:kc * 128 + ksz, :],
                    )
                wiT_sb.append(wi)
                wh = wpool.tile([128, 2 * G4], FP16, tag=f"whT{l}")
                for kc in range(2):
                    nc.sync.dma_start(
                        wh[:, kc * G4:(kc + 1) * G4],
                        whT_d[l][kc * 128:(kc + 1) * 128, :],
                    )
                whT_sb.append(wh)
                bv = wpool.tile([1, G4], FP16, tag=f"bvec{l}")
                nc.sync.dma_start(bv[:], bvec_d[l][:])
                bvec_sb.append(bv)

            ones_sb = wpool.tile([1, VCHUNK], FP16, tag="ones")
            nc.vector.memset(ones_sb[:], 1.0)
            ident = wpool.tile([128, 128], FP32, tag="ident")
            make_identity(nc, ident[:])
            zrhs = wpool.tile([128, B_LOC], FP16, tag="zrhs")
            nc.vector.memset(zrhs[:], 0.0)

            # h buffers: hbuf[l] [128, 2*tb] fp16, col = kc*tb + t*B_LOC + b
            # layer "-1" input = xT from embedding (K chunks 128 + 72)
            xT = spool.tile([128, 2 * tb], FP16, tag="xT", name="xT")
            hbuf = [spool.tile([128, 2 * tb], FP16, tag=f"h{l}", name=f"h{l}")
                    for l in range(3)]

            # ---- Phase 1: embedding gather + transpose ----
            for gt in range(n_gtiles):
                p = min(128, ntok - gt * 128)
                idt = work.tile([128, 1], mybir.dt.int32, tag="ids")
                nc.sync.dma_start(idt[0:p, :], xids_d[gt * 128:gt * 128 + p, :])
                gat = work.tile([128, E], FP32, tag="gather")
                nc.gpsimd.indirect_dma_start(
                    out=gat[0:p, :],
                    out_offset=None,
                    in_=emb_d[:, :],
                    in_offset=bass.IndirectOffsetOnAxis(ap=idt[0:p, :1], axis=0),
                )
                for kc, ksz in enumerate(ksizes(E)):
                    tp = fcpp.tile([128, VCHUNK], FP32, tag="fcpsum")
                    nc.tensor.transpose(
                        tp[0:ksz, 0:p], gat[0:p, kc * 128:kc * 128 + ksz],
                        ident[0:p, 0:p],
                    )
                    nc.vector.tensor_copy(
                        xT[0:ksz, kc * tb + gt * 128:kc * tb + gt * 128 + p],
                        tp[0:ksz, 0:p],
                    )

            # ---- Phase 2: LSTM layers ----
            for l in range(3):
                ks = ksizes(LAYER_DIMS[l])
                src = xT if l == 0 else hbuf[l - 1]
                dst = hbuf[l]
                wi = wiT_sb[l]
                wh = whT_sb[l]
                cstate = spool.tile([128, 2 * B_LOC], FP32, tag="cstate")
                nc.vector.memset(cstate[:], 0.0)

                for ch in range(nchunks):
                    t0 = ch * chunk
                    # gate psum for this chunk: [128, 8*cb] fp32 (2 banks)
                    gp = pp.tile([128, N_MCHUNK * cb], FP32, tag="gates")
                    gp3 = gp[:].rearrange("p (m c) -> p m c", m=N_MCHUNK)
                    # xg precompute: gp[m] = Wi^T-chunk @ src-chunk + bias.
                    # start=True only on the first matmul issued into each
                    # 2KB PSUM bank (it marks the whole bank pending-zero, so
                    # every element's first write then auto-overwrites);
                    # explicit deps keep the bank opener first since other
                    # m-chunks don't overlap its region.
                    bank_openers = {}
                    for m in range(N_MCHUNK):
                        bank = (m * cb * 4) // 2048
                        for kc, ksz in enumerate(ks):
                            is_open = kc == 0 and bank not in bank_openers
                            mm = nc.tensor.matmul(
                                gp[:, m * cb:(m + 1) * cb],
                                lhsT=wi[0:ksz, kc * G4 + m * 128:kc * G4 + (m + 1) * 128],
                                rhs=src[0:ksz, kc * tb + t0 * B_LOC:kc * tb + (t0 + chunk) * B_LOC],
                                start=is_open,
                                stop=False,
                                skip_group_check=True,
                            )
                            if is_open:
                                bank_openers[bank] = mm.ins
                            elif kc == 0:
                                add_dep_helper(
                                    mm.ins, bank_openers[bank], sync=False,
                                    reason="psum bank opener ordering",
                                )
                        nc.tensor.matmul(
                            gp[:, m * cb:(m + 1) * cb],
                            lhsT=bvec_sb[l][:, m * 128:(m + 1) * 128],
                            rhs=ones_sb[:, 0:cb],
                            start=False,
                            stop=False,
                            skip_group_check=True,
                        )
                    # recurrence steps
                    for ts in range(chunk):
                        t = t0 + ts
                        for kc in range(2):
                            if t == 0:
                                rhs = zrhs[:, 0:B_LOC]
                            else:
                                rhs = dst[:, kc * tb + (t - 1) * B_LOC:kc * tb + t * B_LOC]
                            for m in range(N_MCHUNK):
                                nc.tensor.matmul(
                                    gp[:, m * cb + ts * B_LOC:m * cb + (ts + 1) * B_LOC],
                                    lhsT=wh[:, kc * G4 + m * 128:kc * G4 + (m + 1) * 128],
                                    rhs=rhs,
                                    start=False,
                                    stop=(kc == 1),
                                    skip_group_check=True,
                                )
                        # gates: chunks 0-5 sigmoid (i0,i1,f0,f1,o0,o1), 6-7 tanh (g)
                        sig = work.tile([128, 6 * B_LOC], FP32, tag="sig")
                        sig3 = sig[:].rearrange("p (m c) -> p m c", m=6)
                        nc.scalar.activation(
                            sig3, gp3[:, 0:6, ts * B_LOC:(ts + 1) * B_LOC], AF.Sigmoid
                        )
                        gg = work.tile([128, 2 * B_LOC], FP32, tag="gg")
                        gg3 = gg[:].rearrange("p (m c) -> p m c", m=2)
                        nc.scalar.activation(
                            gg3, gp3[:, 6:8, ts * B_LOC:(ts + 1) * B_LOC], AF.Tanh
                        )
                        # c = f*c + i*g ; h = o*tanh(c)
                        t1 = work.tile([128, 2 * B_LOC], FP32, tag="t1")
                        t2 = work.tile([128, 2 * B_LOC], FP32, tag="t2")
                        nc.vector.tensor_mul(t1[:], sig[:, 2 * B_LOC:4 * B_LOC], cstate[:])
                        nc.vector.tensor_mul(t2[:], sig[:, 0:2 * B_LOC], gg[:])
                        nc.vector.tensor_add(cstate[:], t1[:], t2[:])
                        tcs = work.tile([128, 2 * B_LOC], FP32, tag="tc")
                        nc.scalar.activation(tcs[:], cstate[:], AF.Tanh)
                        hview = dst[:].rearrange("p (k c) -> p k c", k=2)
                        nc.vector.tensor_mul(
                            hview[:, :, t * B_LOC:(t + 1) * B_LOC],
                            sig[:, 4 * B_LOC:6 * B_LOC].rearrange("p (k c) -> p k c", k=2),
                            tcs[:].rearrange("p (k c) -> p k c", k=2),
                        )

            # ---- Phase 3: FC + log_softmax (2 passes) ----
            n_m = (tb + 127) // 128
            n_v = (V + VCHUNK - 1) // VCHUNK
            zacc = spool.tile([128, n_m * n_v], FP32, tag="zacc")
            neglse = spool.tile([128, n_m], FP32, tag="neglse")
            h2 = hbuf[2]

            for pass_i in range(2):
                for v in range(n_v):
                    vs = v * VCHUNK
                    vsz = min(VCHUNK, V - vs)
                    fw = fcwpool.tile([128, 2 * VCHUNK], FP16, tag="fcw")
                    for kc in range(2):
                        nc.sync.dma_start(
                            fw[:, kc * VCHUNK:kc * VCHUNK + vsz],
                            fcWT_d[kc * 128:(kc + 1) * 128, vs:vs + vsz],
                        )
                    fb = fcwpool.tile([1, VCHUNK], FP16, tag="fcb")
                    nc.sync.dma_start(fb[:, 0:vsz], fcb_d[:, vs:vs + vsz])
                    for m in range(n_m):
                        msz = min(128, tb - m * 128)
                        ps = fcpp.tile([128, VCHUNK], FP32, tag="fcpsum")
                        for kc in range(2):
                            nc.tensor.matmul(
                                ps[0:msz, 0:vsz],
                                lhsT=h2[:, kc * tb + m * 128:kc * tb + m * 128 + msz],
                                rhs=fw[:, kc * VCHUNK:kc * VCHUNK + vsz],
                                start=(kc == 0),
                                stop=False,
                                skip_group_check=True,
                            )
                        nc.tensor.matmul(
                            ps[0:msz, 0:vsz],
                            lhsT=ones_sb[:, 0:msz],
                            rhs=fb[:, 0:vsz],
                            start=False,
                            stop=True,
                            skip_group_check=True,
                        )
                        if pass_i == 0:
                            esc = stpool.tile([128, VCHUNK], FP32, tag="expsc")
                            nc.scalar.activation(
                                esc[0:msz, 0:vsz], ps[0:msz, 0:vsz], AF.Exp,
                                accum_out=zacc[0:msz, m * n_v + v:m * n_v + v + 1],
                            )
                        else:
                            st = stpool.tile([128, VCHUNK], FP32, tag="stage")
                            if m % 2 == 0:
                                nc.scalar.activation(
                                    st[0:msz, 0:vsz], ps[0:msz, 0:vsz], AF.Identity,
                                    bias=neglse[0:msz, m:m + 1],
                                )
                            else:
                                nc.vector.tensor_scalar_add(
                                    st[0:msz, 0:vsz], ps[0:msz, 0:vsz],
                                    neglse[0:msz, m:m + 1],
                                )
                            nc.sync.dma_start(
                                out_d[m * 128:m * 128 + msz, vs:vs + vsz],
                                st[0:msz, 0:vsz],
                            )
                if pass_i == 0:
                    # lse per m-chunk: neglse = -ln(sum(zacc))
                    for m in range(n_m):
                        msz = min(128, tb - m * 128)
                        zs = work.tile([128, 1], FP32, tag="zsum")
                        nc.vector.tensor_reduce(
                            zs[0:msz], zacc[0:msz, m * n_v:(m + 1) * n_v],
                            op=mybir.AluOpType.add, axis=mybir.AxisListType.X,
                        )
                        lse = work.tile([128, 1], FP32, tag="lse")
                        nc.scalar.activation(lse[0:msz], zs[0:msz], AF.Ln)
                        nc.vector.tensor_scalar_mul(
                            neglse[0:msz, m:m + 1], lse[0:msz], -1.0)

    nc.compile()
    return nc


_nc_cache = {}


def _get_nc(t_steps):
    if t_steps not in _nc_cache:
        _nc_cache[t_steps] = build_nc(t_steps)
    return _nc_cache[t_steps]


def prep_inputs(x, emb, Wi, Wh, bb, fcW, fcb, t_steps=T):
    """Host-side shard + repack. Returns in_maps for the 8 cores."""
    perm = np.concatenate([np.arange(0, 512), np.arange(768, 1024),
                           np.arange(512, 768)])  # i,f | o | g
    shared = {
        "emb": np.ascontiguousarray(emb.astype(np.float32)),
        "fcWT": np.ascontiguousarray(fcW.T.astype(np.float16)),
        "fcb": np.ascontiguousarray(fcb[None, :].astype(np.float16)),
    }
    for l in range(3):
        shared[f"wiT{l}"] = np.ascontiguousarray(Wi[l][perm].T.astype(np.float16))
        shared[f"whT{l}"] = np.ascontiguousarray(Wh[l][perm].T.astype(np.float16))
        shared[f"bvec{l}"] = np.ascontiguousarray(bb[l][perm][None, :].astype(np.float16))
    in_maps = []
    for c in range(N_CORES):
        x_loc = x[c * B_LOC:(c + 1) * B_LOC, :t_steps]  # [B_LOC, t]
        xids = np.ascontiguousarray(
            x_loc.T.reshape(-1, 1).astype(np.int32))  # [(t b), 1]
        m = dict(shared)
        m["xids"] = xids
        in_maps.append(m)
    return in_maps


def kernel(x, emb, Wi0, Wh0, b0, Wi1, Wh1, b1, Wi2, Wh2, b2, fcW, fcb,
           t_steps=T, trace=False):
    x = np.asarray(x)
    nc = _get_nc(t_steps)
    in_maps = prep_inputs(
        np.asarray(x), np.asarray(emb),
        [np.asarray(Wi0), np.asarray(Wi1), np.asarray(Wi2)],
        [np.asarray(Wh0), np.asarray(Wh1), np.asarray(Wh2)],
        [np.asarray(b0), np.asarray(b1), np.asarray(b2)],
        np.asarray(fcW), np.asarray(fcb), t_steps)
    res = run_bass_kernel_spmd(nc, in_maps, core_ids=list(range(N_CORES)),
                               trace=trace)
    out = np.empty((B, t_steps, V), np.float32)
    for c in range(N_CORES):
        oc = res.results[c]["out"].reshape(t_steps, B_LOC, V)
        out[c * B_LOC:(c + 1) * B_LOC] = oc.transpose(1, 0, 2)
    kernel.last_results = res
    return out
